# revision 35
# baseline (speedup 1.0000x reference)
"""AgreementRouter (3-iter dynamic routing) on 8 trn2 cores — v16 (91.4us,
from 116.3us baseline).

Math: logits L[b,n,c] (init 0); per iter: a = softmax_c(L);
o[c,f] = sum_n a[n,c] x[n,c,f] + bias; if not last: L += sum_f x[n,c,f] o[c,f].
Return final o.  B=64 (8/core), N=1152, C=32, F=16, CF=512 (cf = f*32+c).

Key decisions (each verified against a perfetto trace on hw):
  - xn DRAM layout [b][128p][9t][512cf]: 9KB contiguous per partition line
    (1KB descriptors halved effective DMA rate in the v11 layout).
  - All x loads on the sync HWDGE ring, ordered xn0,xt0,xn1,... via
    sync=False scheduling deps; ordered arrival at full rate, no gating.
  - DMA streams 18.9MB at ~420GB/s, done ~51.5us; PE tracks it.
  - L2 = x.(o0+o1) in ONE g-pass (weights encode o0+o1+bias): removes the
    L1 SBUF copy + re-add from the critical softmax chain.  (PSUM accum
    groups canNOT be resumed/interleaved across other instructions — HW
    produces wrong results; each (t,k) accumulation run stays contiguous.)
  - PSUM tiles padded to full 2KB banks: sub-bank slot packing made the
    BankOverlapTracker serialize PE-writes vs DVE-reads across slots
    (cost ~2-8us and large run-to-run variance).
  - S-phase tail on scalar+gpsimd (PSUM scaled copy, then adds): the
    vector queue is FIFO, and S's vector ops (waiting on the wave's last
    PE matmuls) head-of-line blocked the next wave's softmax.
  - Per-wave emission order [sm1, o1, g2+sm2, o2, g1, S]: vector leads
    with ready work; sm2(b+1) overlaps o2mm(b) on the PE.
  - Output stored untransposed ([128p, 8b*4k] f32, one DMA); host
    transposes.  SBUF pools use pool_alloc_mode="queue" (ring over free
    SBUF; equal-or-better vs stack in interleaved A/B).
  - Known residual overheads: ~6us framework preamble, ~8us end-of-kernel
    EVSEM barrier, 3x ~2.5us PE instruction-page fetch stalls (pc multiples
    of 256; fetches contend with the saturated x stream), drain chains for
    the last 2 b's.  Floor with this architecture ~ 80us.

Per-core dataflow (all five x-passes on the PE, x enters the weight port):
  - S-pass:  lhsT = xn chunk [128n,128cf], rhs = ones [128,1] -> S_col [128cf,4k]
  - o-pass:  lhsT = xn chunk,              rhs = a   [128,32] -> oT [128cf,3c,4k,32c']
             diag extract: oT * maskT, reduce over c' then chunk -> o_col [128cf,4]
  - g-pass:  lhsT = xt chunk [128cf,128n], rhs = W   [128,32] -> L [128n,9t,32c]
             W = maskT * (o_col + bias_col) (fp16; mixed bf16xfp16 matmul)
  - softmax1 skips max-subtraction (|L1| < 40); softmax2 subtracts the max.
"""

import sys

sys.path.insert(0, "/opt/trn_rl_repo")

import numpy as np
import ml_dtypes

import concourse.bass as bass
import concourse.bacc as bacc
import concourse.tile as tile
from concourse import mybir
from concourse.tile_rust import add_dep_helper

B, N, C, F = 64, 1152, 32, 16
CF = C * F          # 512
P = 128
NT = N // P         # 9
NCH = CF // P       # 4
NCORES = 8
BLOC = B // NCORES  # 8
import os
NCK = 1             # softmax/o-pass chunks (stream b's)
NCKTAIL = 1         # chunks for the last 2 b's
NCKB = [NCK] * (BLOC - 2) + [NCKTAIL, NCKTAIL]
TCK = NT // NCK

F32 = mybir.dt.float32
F16 = mybir.dt.float16
BF16 = mybir.dt.bfloat16
AX_X = mybir.AxisListType.X
MUL = mybir.AluOpType.mult
ADD = mybir.AluOpType.add


def build_bass(compile=True):
    nc = bacc.Bacc("TRN2")

    xn_dram = nc.dram_tensor("xn", [BLOC, P, NT, CF], BF16, kind="ExternalInput")
    xt_dram = nc.dram_tensor("xt", [BLOC, P, NCH * N], BF16, kind="ExternalInput")
    maskT_dram = nc.dram_tensor("maskT", [P, C], F16, kind="ExternalInput")
    biascol_dram = nc.dram_tensor("biascol", [P, NCH], F32, kind="ExternalInput")
    out_dram = nc.dram_tensor("out", [P, BLOC * NCH], F32, kind="ExternalOutput")

    with tile.TileContext(nc, pool_alloc_mode="queue") as tc:
        with (
            tc.tile_pool(name="xn", bufs=1) as xn_pool,
            tc.tile_pool(name="xt", bufs=1) as xt_pool,
            tc.tile_pool(name="consts", bufs=1) as consts,
            tc.tile_pool(name="wpool", bufs=3) as wpool,
            tc.tile_pool(name="grp", bufs=6) as grp,
            tc.tile_pool(name="smal", bufs=8) as smal,
            tc.tile_pool(name="ps_L", bufs=4, space="PSUM") as ps_L,
            tc.tile_pool(name="ps_o", bufs=2, space="PSUM") as ps_o,
        ):
            # ---------- constants (scalar HWDGE ring) ----------
            ones_col = consts.tile([P, 1], BF16, tag="ones", name="ones")
            nc.vector.memset(ones_col, 1.0)
            maskT = consts.tile([P, C], F16, tag="maskT", name="maskT")
            nc.scalar.dma_start(out=maskT, in_=maskT_dram[:])
            bias_col = consts.tile([P, NCH], F32, tag="biascol", name="biascol")
            nc.scalar.dma_start(out=bias_col, in_=biascol_dram[:])
            osb = consts.tile([P, BLOC, NCH], F32, tag="osb", name="osb")

            # ---------- x loads: sync HWDGE ring, ordered xn0,xt0,xn1,... ----------
            xn = [None] * BLOC
            xt = [None] * BLOC
            prev_load = None
            for b in range(BLOC):
                t_xn = xn_pool.tile([P, NT, CF], BF16, tag=f"xn{b}", name=f"xn{b}")
                i1 = nc.sync.dma_start(out=t_xn, in_=xn_dram[b])
                if prev_load is not None:
                    add_dep_helper(i1.ins, prev_load.ins, sync=False, reason="order")
                t_xt = xt_pool.tile([P, NCH, N], BF16, tag=f"xt{b}", name=f"xt{b}")
                i2 = nc.sync.dma_start(
                    out=t_xt, in_=xt_dram[b].rearrange("p (k n) -> p k n", n=N)
                )
                add_dep_helper(i2.ins, i1.ins, sync=False, reason="order")
                prev_load = i2
                xn[b] = t_xn
                xt[b] = t_xt

            # persistent per-b state
            Lps = [None] * BLOC     # L psum [P, NT, C] f32
            bc0 = [None] * BLOC     # o0 + bias [P, NCH] f32
            a_t = [None] * BLOC     # list of NCK chunk tiles fp16 [P, TCK, C]
            W_t = [None] * BLOC     # W fp16 [P, NCH, C]

            # ---------- phase S: column sums -> W0 ----------
            def phase_S(b):
                sps = ps_o.tile([P, 4, NCH, C], F32, tag="ot", name=f"s{b}")[:, :NCK]
                for k in range(NCH):
                    for t in range(NT):
                        nc.tensor.matmul(
                            sps[:, 0, k, 0:1],
                            lhsT=xn[b][:, t, k * P : (k + 1) * P],
                            rhs=ones_col,
                            start=(t == 0),
                            stop=(t == NT - 1),
                        )
                osc = smal.tile([P, NCH], F32, tag="osc", name=f"osc{b}")
                nc.scalar.mul(osc, sps[:, 0, :, 0], 1.0 / C)
                oc = smal.tile([P, NCH], F32, tag="ocol", name=f"ocol0_{b}")
                nc.gpsimd.tensor_tensor(oc, osc, bias_col, ADD)
                w = wpool.tile([P, NCH, C], F16, tag=f"w{b}", name=f"w0_{b}")
                nc.gpsimd.tensor_tensor(
                    w,
                    oc[:, :, None].to_broadcast([P, NCH, C]),
                    maskT[:, None, :].to_broadcast([P, NCH, C]),
                    MUL,
                )
                W_t[b] = w
                # bc0 = o0 + bias (so the o1-extract can produce o0+o1+bias
                # in one add: g2 weights encode L2 = x.(o0+o1) directly)
                bc = smal.tile([P, NCH], F32, tag="bc0", name=f"bc0_{b}")
                nc.gpsimd.tensor_tensor(bc, oc, bias_col, ADD)
                bc0[b] = bc

            # ---------- g-pass: 36 matmuls -> L [P, NT, C] psum ----------
            def phase_g(b, first):
                Lps[b] = ps_L.tile([P, 16, C], F32, tag="L", name=f"L{b}{first}")[:, :NT]
                lt = Lps[b]
                for t in range(NT):
                    for k in range(NCH):
                        nc.tensor.matmul(
                            lt[:, t, :],
                            lhsT=xt[b][:, k, t * P : (t + 1) * P],
                            rhs=W_t[b][:, k, :],
                            start=(k == 0),
                            stop=(k == NCH - 1),
                        )

            # ---------- softmax over c (chunked over t) ----------
            def phase_softmax(b, first):
                nck = NCKB[b]
                tck = NT // nck
                ags = []
                for c in range(nck):
                    ts = slice(c * tck, (c + 1) * tck)
                    if first:
                        # |L1| < 40: exp in f32, no max subtraction; e in bf16
                        e = grp.tile([P, tck, C], BF16, tag=f"e1{c}", name=f"e{b}1{c}")
                        nc.scalar.activation(
                            out=e,
                            in_=Lps[b][:, ts],
                            func=mybir.ActivationFunctionType.Exp,
                        )
                    else:
                        negmax = smal.tile(
                            [P, tck], F32, tag=f"nm{c}", name=f"nm{b}{c}"
                        )
                        nc.vector.reduce_max(negmax, Lps[b][:, ts], axis=AX_X, negate=True)
                        el = grp.tile([P, tck, C], F16, tag=f"el{c}", name=f"el{b}{c}")
                        nc.vector.tensor_tensor(
                            el,
                            Lps[b][:, ts],
                            negmax[:, :, None].to_broadcast([P, tck, C]),
                            ADD,
                        )
                        e = grp.tile([P, tck, C], F16, tag=f"e2{c}", name=f"e{b}2{c}")
                        nc.scalar.activation(
                            out=e, in_=el, func=mybir.ActivationFunctionType.Exp
                        )
                    z = smal.tile([P, tck], F32, tag=f"z{c}", name=f"z{b}{first}{c}")
                    nc.vector.reduce_sum(z, e, axis=AX_X)
                    rz = smal.tile(
                        [P, tck],
                        BF16 if first else F16,
                        tag=f"rz{c}",
                        name=f"rz{b}{first}{c}",
                    )
                    with nc.allow_low_precision(reason="1/Z scale, tiny"):
                        nc.vector.reciprocal(rz, z)
                    ag = grp.tile(
                        [P, tck, C], F16, tag=f"ag{c}", name=f"a{b}{first}{c}"
                    )
                    nc.vector.tensor_tensor(
                        ag, e, rz[:, :, None].to_broadcast([P, tck, C]), MUL
                    )
                    ags.append(ag)
                a_t[b] = ags

            # ---------- o-pass: chunked matmuls + diag extract ----------
            def phase_o(b, final):
                nck = NCKB[b]
                tck = NT // nck
                ots = ps_o.tile([P, 4, NCH, C], F32, tag="ot", name=f"ot{b}{final}")[:, :nck]
                for c in range(nck):
                    for k in range(NCH):
                        for i in range(tck):
                            t = c * tck + i
                            nc.tensor.matmul(
                                ots[:, c, k, :],
                                lhsT=xn[b][:, t, k * P : (k + 1) * P],
                                rhs=a_t[b][c][:, i, :],
                                start=(i == 0),
                                stop=(i == tck - 1),
                            )
                msk = grp.tile([P, nck, NCH, C], F32, tag="msk", name=f"msk{b}{final}")
                nc.vector.tensor_tensor(
                    msk,
                    ots,
                    maskT[:, None, None, :].to_broadcast([P, nck, NCH, C]),
                    MUL,
                )
                ock = smal.tile([P, nck, NCH], F32, tag="ock", name=f"ock{b}{final}")
                nc.vector.reduce_sum(ock, msk, axis=AX_X)
                if nck == 1:
                    ocr = ock[:, 0]
                else:
                    ocr = smal.tile([P, NCH], F32, tag="ocr", name=f"ocr{b}{final}")
                    nc.vector.reduce_sum(
                        ocr, ock.rearrange("p c k -> p k c"), axis=AX_X
                    )
                if final:
                    nc.gpsimd.tensor_tensor(osb[:, b], ocr, bias_col, ADD)
                else:
                    # oc = o0 + o1 + bias: g2 weights encode L2 = x.(o0+o1)
                    oc = smal.tile([P, NCH], F32, tag="ocol", name=f"ocol{b}{final}")
                    nc.gpsimd.tensor_tensor(oc, ocr, bc0[b], ADD)
                    w = wpool.tile([P, NCH, C], F16, tag=f"w{b}", name=f"w1_{b}")
                    nc.gpsimd.tensor_tensor(
                        w,
                        oc[:, :, None].to_broadcast([P, NCH, C]),
                        maskT[:, None, :].to_broadcast([P, NCH, C]),
                        MUL,
                    )
                    W_t[b] = w

            # ---------- software-pipelined wavefront ----------
            # p2 (sm1 of b=w-2) emitted first each wave: its deps (g1 of
            # wave w-1) are done, so the in-order vector queue leads with
            # ready work.
            PH = [
                phase_S,
                lambda b: phase_g(b, first=True),
                lambda b: phase_softmax(b, first=True),
                lambda b: phase_o(b, final=False),
                lambda b: (phase_g(b, first=False), phase_softmax(b, first=False)),
                lambda b: phase_o(b, final=True),
            ]
            NPH = len(PH)
            OFF = 1  # phase stagger
            # per-wave emission order: sm1 (ready vector work first), o1,
            # g2+sm2, o2 (so sm2(b+1) overlaps o2mm(b) on the PE), g1, S
            EMIT = [2, 3, 4, 5, 1, 0]
            for w in range(NPH + OFF * (BLOC - 1)):
                for ph in EMIT:
                    if (w - ph) % OFF == 0:
                        b = (w - ph) // OFF
                        if 0 <= b < BLOC:
                            PH[ph](b)

            nc.scalar.dma_start(
                out=out_dram[:], in_=osb.rearrange("p b k -> p (b k)")
            )

    if compile:
        nc.compile()
    return nc


_NC_CACHE = None


def _get_nc():
    global _NC_CACHE
    if _NC_CACHE is None:
        _NC_CACHE = build_bass()
    return _NC_CACHE


def _make_consts():
    # cf index = f*C + c  (f outer, c inner); chunk k covers cf in [128k, 128k+128)
    p = np.arange(P)
    maskT = np.zeros((P, C), dtype=np.float16)
    maskT[p, p % C] = 1.0
    k = np.arange(NCH)
    cfs = k[None, :] * P + p[:, None]  # [P, NCH] global cf index
    return maskT, cfs


def _install_ntff_hook():
    import contextlib
    import ctypes
    import types

    if "antenv.axon_hooks" in sys.modules:
        return
    try:
        from antenv.axon_hooks import get_axon_ntff_profile_hook  # noqa: F401

        return
    except ImportError:
        pass

    so_path = "/opt/axon/libaxon_pjrt.so"
    try:
        lib = ctypes.CDLL(so_path)
    except OSError:
        return
    if not hasattr(lib, "axon_start_nrt_profile"):
        return
    lib.axon_start_nrt_profile.argtypes = [
        ctypes.POINTER(ctypes.c_int64),
        ctypes.c_size_t,
    ]
    lib.axon_start_nrt_profile.restype = ctypes.c_int64
    lib.axon_stop_nrt_profile.argtypes = [ctypes.c_char_p]
    lib.axon_stop_nrt_profile.restype = ctypes.c_int64

    @contextlib.contextmanager
    def _hook(output_dir, device_ids):
        import jax

        jax.devices()
        if device_ids:
            ids = (ctypes.c_int64 * len(device_ids))(*device_ids)
            rc = lib.axon_start_nrt_profile(ids, len(device_ids))
        else:
            rc = lib.axon_start_nrt_profile(None, 0)
        if rc != 0:
            raise RuntimeError(f"axon_start_nrt_profile rc={rc}")
        try:
            yield
        finally:
            n = lib.axon_stop_nrt_profile(str(output_dir).encode())
            print(f"profile: {n} file(s) written to {output_dir}")

    mod = types.ModuleType("antenv.axon_hooks")
    mod.get_axon_ntff_profile_hook = lambda: _hook
    mod.set_axon_ntff_profile_hook = lambda h: None
    sys.modules["antenv.axon_hooks"] = mod


def _run(inputs, bias, trace=False):
    import concourse.bass_utils as bu
    from concourse.bass_utils import run_bass_kernel_spmd

    if trace:
        _install_ntff_hook()
        bu.upload_artifacts = lambda tmpdir: tmpdir

    bf = ml_dtypes.bfloat16
    # device order: cf = f*C + c
    x = np.ascontiguousarray(
        np.asarray(inputs, dtype=np.float32).reshape(B, N, C, F).transpose(0, 1, 3, 2)
    ).reshape(B, N, CF)
    x16 = x.astype(bf)
    # xn layout: [b][p][t][cf]  (per-partition 9KB contiguous)
    xn = np.ascontiguousarray(x16.reshape(B, NT, P, CF).transpose(0, 2, 1, 3))
    # transposed copy: xt[b][p, k*N + n] = x[b, n, 128k+p]
    xt = np.ascontiguousarray(
        x16.reshape(B, N, NCH, P).transpose(0, 3, 2, 1)
    ).reshape(B, P, NCH * N)

    bias_f = np.asarray(bias, dtype=np.float32).T.reshape(CF)  # (f, c) order
    maskT, cfs = _make_consts()
    bias_col = bias_f[cfs].astype(np.float32)  # [P, NCH]

    in_maps = [
        {
            "xn": xn[i * BLOC : (i + 1) * BLOC],
            "xt": xt[i * BLOC : (i + 1) * BLOC],
            "maskT": maskT,
            "biascol": np.ascontiguousarray(bias_col),
        }
        for i in range(NCORES)
    ]
    nc = _get_nc()
    res = run_bass_kernel_spmd(nc, in_maps, core_ids=list(range(NCORES)), trace=trace)
    # out[p, b, k] = o_b[cf = k*128 + p]  ->  [b, cf] -> [b, C, F]
    out = np.concatenate(
        [
            r["out"]
            .reshape(P, BLOC, NCH)
            .transpose(1, 2, 0)
            .reshape(BLOC, CF)
            .reshape(BLOC, F, C)
            .transpose(0, 2, 1)
            for r in res.results
        ],
        axis=0,
    )
    return out.astype(np.float32), res


def kernel(**inputs) -> np.ndarray:
    out, _ = _run(inputs["inputs"], inputs["bias"], trace=False)
    return out


def kernel_traced(**inputs):
    out, res = _run(inputs["inputs"], inputs["bias"], trace=True)
    return out, res
